# revision 5
# baseline (speedup 1.0000x reference)
"""Trainium2 Bass kernel for nn_DifferentiableFDN.

Math: the module is linear in x, so
    out[b,t] = sum_j w_j * y_j[b,t],   w = (H^T alpha + beta)/16,
    y_j = first-order IIR of x with decay a_j.

Blocked-scan scheme (chunk length L=128, NCH=375 chunks per batch row).
The host pre-transposes x into XT[b] = (t=128, c=375) and un-transposes the
output. All matmul operands are bf16 (PSUM accumulates fp32); the chunk-carry
scan state stays fp32 inside the DVE. Per batch row b:
  - e  = P^T  @ XT   (16 x 375)   chunk-end state contributions, four
         matmuls in disjoint PE column quadrants run concurrently
  - S  : ONE tensor_tensor_scan over the stacked tile,
         S[c] = a_j^L S[c-1] + e[c], written bf16 shifted (ssh[c] = S[c-1])
  - z  = MT^T @ XT   (128 x 375)  local Toeplitz part (start=True zeroes
         the whole bank row, so z is ONE matmul per bank)
  - z += Wc^T @ ssh  rank-16 carry correction, split into column halves
         (A = chunks 0:188, B = 188:375) in disjoint row quadrants so the
         A-half staging casts and output DMA launch while B still runs
  out[b, c*128+tp] = z[tp, c], cast bf16 into half-major staging tiles,
  A-half DMA on the sync queue / B-half on the scalar queue, host converts.

DMA plan (from ntff packet analysis): per-queue cost = SEQ ~0.6us + DGE
start ~0.65us + ~7-9ns per 1500B descriptor + completion-semaphore straggle
(16 per-engine increments, ~30ns apart, skew follows the last packets).
So: one bulk x transfer per HWDGE queue with a small 16-row TAIL DMA split
off so the 16 completion sems land compactly; the const pack (with the f32
scan multiplier byte-packed into two bf16 columns) rides the sync queue
after x.  SWDGE is NOT used for anything gating compute (its completion
sems trail packets by >1us).

PE clock: p-states top out at ~1.2GHz (0.83ns/col) for this kernel no
matter how long the warm-up streak runs (verified: a fully-bridged 6.7us
streak still streams 375-col matmuls in ~310ns).  The warm-up ladder still
pays for itself by lifting the clock from 0.65GHz before x lands.

Sharding: pure data-parallel, 4 batch rows per core x 8 cores.
"""
import numpy as np
import ml_dtypes

B, T = 32, 48000
D = 16
NCORES = 8
BL = B // NCORES            # 4 batch rows per core
L = 128                     # chunk length
NCH = T // L                # 375 chunks per batch row
NA = 188                    # A-half chunks (cols 0:188)
NB = NCH - NA               # B-half chunks (cols 188:375)
NWARM512 = 5                # warm-up ladder: big tiles first ...
NWARM256 = 2                # ... then 256-col quanta
XTAIL = 16                  # tail rows split off each x DMA (compact sems)

_CACHE = {}


def _mirror_f32_params(log_kappa, alpha_raw, beta_raw, H):
    """Reference param math, f64 internally, rounded through f32 where the
    reference's f32 pipeline rounds."""
    sig = 1.0 / (1.0 + np.exp(-log_kappa.astype(np.float64)))
    sig32 = sig.astype(np.float32)
    kappa = (np.float32(1.0) + sig32 * np.float32(799.0)).astype(np.float32)
    inv = (np.float32(-1.0) / kappa).astype(np.float32)
    decays = np.exp(inv.astype(np.float64)).astype(np.float32)
    decays = np.clip(decays, 0.0, 0.9999).astype(np.float64)
    alpha = (1.0 / (1.0 + np.exp(-alpha_raw.astype(np.float64))))
    beta = (1.0 / (1.0 + np.exp(-beta_raw.astype(np.float64))))
    alpha = alpha.astype(np.float32).astype(np.float64)
    beta = beta.astype(np.float32).astype(np.float64)
    w = (H.astype(np.float64).T @ alpha + beta) / np.float64(D)
    return decays, w


def _tables(decays, w):
    delta = np.arange(L)
    pows = decays[None, :] ** delta[:, None]                   # [L, D] a_j^d
    h = pows @ w                                               # h[d]
    MT = np.zeros((L, L))
    for t in range(L):
        MT[t, t:] = h[: L - t]                                 # MT[t,tp]=h[tp-t]
    P = decays[None, :] ** (L - 1 - delta[:, None])            # [L, D]
    Wc = w[:, None] * decays[:, None] ** (delta[None, :] + 1)  # [D, L]
    bf = ml_dtypes.bfloat16
    # cc = [MT | P | Wc-replicated | mlc-bitcast] (128 x 274) bf16, one DMA.
    # The 4 batch rows' chunk-end states live at PSUM partition offsets
    # 0/32/64/96 (the only legal PE output tile positions), so the corr
    # weights Wc and the scan multiplier mlc (f32, byte-packed into bf16
    # cols 272:274 -- the scan state is fp32) are replicated at those offsets.
    cc = np.zeros((L, 274), dtype=bf)
    cc[:, 0:128] = MT.astype(bf)
    cc[:, 128:144] = P.astype(bf)
    mlc = np.zeros((L,), dtype=np.float32)
    for b in range(BL):
        cc[32 * b:32 * b + D, 144:272] = Wc.astype(bf)
        mlc[32 * b:32 * b + D] = (decays ** L).astype(np.float32)
    cc[:, 272:274] = mlc.view(np.uint16).reshape(L, 2).view(bf)
    return np.ascontiguousarray(cc)


def _body(tc, oa_ap, ob_ap, x_ap, cc_ap):
    from concourse import mybir
    from contextlib import ExitStack

    nc = tc.nc
    f32 = mybir.dt.float32
    bf16 = mybir.dt.bfloat16

    with ExitStack() as ctx:
        const = ctx.enter_context(tc.tile_pool(name="const", bufs=1))
        xtp = ctx.enter_context(tc.tile_pool(name="xt", bufs=1))
        sshp = ctx.enter_context(tc.tile_pool(name="sshp", bufs=1))
        stgp = ctx.enter_context(tc.tile_pool(name="stg", bufs=1))
        epp = ctx.enter_context(tc.tile_pool(name="e_ps", bufs=1, space="PSUM"))
        zpp = ctx.enter_context(tc.tile_pool(name="z_ps", bufs=1, space="PSUM"))

        cc = const.tile([L, 274], bf16, tag="cc")
        # batch rows are PAIRED per SBUF tile: 1500B partition lines keep the
        # DMA queues at full rate (750B lines run at ~half throughput)
        xtq = [xtp.tile([L, 2 * NCH], bf16, tag=f"xt{q}", name=f"xt{q}")
               for q in range(2)]
        xt = [xtq[b // 2][:, (b % 2) * NCH:(b % 2 + 1) * NCH] for b in range(BL)]
        ssh = sshp.tile([L, NCH], bf16, tag="ssh")
        e_all = epp.tile([L, NCH], f32, tag="e")

        # input DMAs: one bulk transfer per HWDGE queue + 16-row tail for
        # compact completion sems; const pack follows on sync (needed only
        # when the matmuls start, well after it lands).
        XB = L - XTAIL
        nc.sync.dma_start(xtq[0][0:XB, :], x_ap[0:XB, :])
        nc.sync.dma_start(xtq[0][XB:L, :], x_ap[XB:L, :])
        nc.sync.dma_start(cc[:, :], cc_ap[:, :])
        nc.scalar.dma_start(xtq[1][0:XB, :], x_ap[L:L + XB, :])
        nc.scalar.dma_start(xtq[1][XB:L, :], x_ap[L + XB:2 * L, :])

        # scan writes cols 1..NCH-1; col 0 is the zero initial state
        nc.gpsimd.memset(ssh[:, 0:1], 0.0)

        # PE p-state warm-up: dependency-free ladder bridging the preamble
        # to the moment x lands (keeps the clock at ~1.2GHz for the real
        # matmuls instead of 0.65).
        warm_w = nc.const_aps.tensor(1.0, (L, L), bf16)
        warm_x = nc.const_aps.tensor(1.0, (L, 512), bf16)
        wpp = ctx.enter_context(tc.tile_pool(name="w_ps", bufs=1, space="PSUM"))
        w_ps = wpp.tile([L, 512], f32, tag="wps")
        for _ in range(NWARM512):
            nc.tensor.matmul(w_ps[:, :], lhsT=warm_w, rhs=warm_x,
                             start=True, stop=True)
        for _ in range(NWARM256):
            nc.tensor.matmul(w_ps[:, 0:256], lhsT=warm_w, rhs=warm_x[:, 0:256],
                             start=True, stop=True)

        mt_sb, p_sb = cc[:, 0:128], cc[:, 128:144]
        mlc_f32 = cc[:, 272:274].bitcast(f32)    # [L, 1] scan multiplier

        # chunk-end states: 4 matmuls, same stationary P, partition-offset
        # writes (tile positions 0/32/64/96) into one stacked PSUM tile;
        # disjoint column quadrants let all four run concurrently on the PE
        for b in range(BL):
            nc.tensor.matmul(e_all[32 * b:32 * b + D, :], lhsT=p_sb,
                             rhs=xt[b], start=True, stop=True,
                             skip_group_check=True, tile_position=(0, 32 * b))

        # ONE scan for all 4 batch rows (DVE cost is per-column, not
        # per-partition); fp32 state internally, bf16 output. The gap
        # partitions carry garbage that nothing reads.
        nc.vector.tensor_tensor_scan(
            ssh[:, 1:NCH], data0=mlc_f32[:, 0:1].broadcast_to((L, NCH - 1)),
            data1=e_all[:, 0:NCH - 1],
            initial=0.0, op0=mybir.AluOpType.mult, op1=mybir.AluOpType.add)

        z = [zpp.tile([L, NCH], f32, tag=f"z{b}", name=f"z{b}")
             for b in range(BL)]
        for b in range(BL):
            nc.tensor.matmul(z[b][:, :], lhsT=mt_sb, rhs=xt[b][:, :],
                             start=True, stop=False, skip_group_check=True)
        # carry correction split into column halves so the A-half staging
        # casts (and output DMA) launch while the B-half still runs; each
        # half is 4 concurrent row-quadrant matmuls.
        for lo, hi, last in ((0, NA, False), (NA, NCH, True)):
            for b in range(BL):
                nc.tensor.matmul(z[b][:, lo:hi],
                                 lhsT=cc[32 * b:32 * b + D, 144:272],
                                 rhs=ssh[32 * b:32 * b + D, lo:hi],
                                 start=False, stop=last, skip_group_check=True,
                                 tile_position=(32 * b, 0))

        # half-major staging: stgA = [b0A|b1A|b2A|b3A], stgB likewise.
        # Casts split DVE (b0, b2) / Act (b1, b3); the A-half output DMA
        # goes on the idle sync engine the moment all A casts land, the
        # B-half issues from Act right after its last cast.
        stgA = stgp.tile([L, BL * NA], bf16, tag="stgA")
        stgB = stgp.tile([L, BL * NB], bf16, tag="stgB")
        for b in range(BL):
            dstA = stgA[:, b * NA:(b + 1) * NA]
            if b % 2:
                nc.scalar.copy(dstA, z[b][:, 0:NA])
            else:
                nc.vector.tensor_copy(dstA, z[b][:, 0:NA])
        for b in range(BL):
            dstB = stgB[:, b * NB:(b + 1) * NB]
            if b % 2:
                nc.scalar.copy(dstB, z[b][:, NA:NCH])
            else:
                nc.vector.tensor_copy(dstB, z[b][:, NA:NCH])
        nc.sync.dma_start(oa_ap[:, :], stgA[:, :])
        nc.scalar.dma_start(ob_ap[:, :], stgB[:, :])


def _build(num_devices=NCORES):
    import concourse.tile as tile
    from concourse import bacc, mybir

    bf16 = mybir.dt.bfloat16
    nc = bacc.Bacc("TRN2", target_bir_lowering=False, debug=False,
                   num_devices=num_devices)
    # x rows 0..127 = queue 0 (b0|b1 column-paired), rows 128..255 = queue 1
    x_ap = nc.dram_tensor("x", [2 * L, 2 * NCH], bf16, kind="ExternalInput").ap()
    cc_ap = nc.dram_tensor("cc", [L, 274], bf16, kind="ExternalInput").ap()
    # out halves: oa[tp, b*NA + c], ob[tp, b*NB + (c-NA)]
    oa_ap = nc.dram_tensor("oa", [L, BL * NA], bf16, kind="ExternalOutput").ap()
    ob_ap = nc.dram_tensor("ob", [L, BL * NB], bf16, kind="ExternalOutput").ap()

    with tile.TileContext(nc) as tc:
        _body(tc, oa_ap, ob_ap, x_ap, cc_ap)
    nc.compile()
    return nc


def _in_maps(x, log_kappa, alpha_raw, beta_raw, H):
    decays, w = _mirror_f32_params(np.asarray(log_kappa), np.asarray(alpha_raw),
                                   np.asarray(beta_raw), np.asarray(H))
    cc = _tables(decays, w)
    bf = ml_dtypes.bfloat16
    x = np.asarray(x, dtype=np.float32)
    # host pre-transpose: (B, T) -> per-core (2*L, 2*NCH) with batch rows
    # column-paired per DMA queue, bf16
    xt_all = x.reshape(B, NCH, L).transpose(0, 2, 1).astype(bf)  # (B, L, NCH)
    maps = []
    for c in range(NCORES):
        quad = xt_all[c * BL:(c + 1) * BL]           # (4, L, NCH)
        xs = quad.reshape(2, 2, L, NCH).transpose(0, 2, 1, 3).reshape(
            2 * L, 2 * NCH)                          # row q*L+p, col b*NCH+c
        maps.append({"x": np.ascontiguousarray(xs), "cc": cc})
    return maps


def _gather(results):
    # oa (L, BL*NA) + ob (L, BL*NB) -> (BL, T), t = c*L + tp
    outs = []
    for c in range(NCORES):
        a = np.asarray(results[c]["oa"]).reshape(L, BL, NA)
        bb = np.asarray(results[c]["ob"]).reshape(L, BL, NB)
        arr = np.concatenate([a, bb], axis=2)        # (L, BL, NCH)
        outs.append(arr.transpose(1, 2, 0).reshape(BL, T))
    return np.concatenate(outs, axis=0).astype(np.float32)


def kernel(x, log_kappa, alpha_raw, beta_raw, H):
    from concourse import bass_utils

    if "nc" not in _CACHE:
        _CACHE["nc"] = _build()
    nc = _CACHE["nc"]
    maps = _in_maps(x, log_kappa, alpha_raw, beta_raw, H)
    res = bass_utils.run_bass_kernel_spmd(nc, maps, core_ids=list(range(NCORES)))
    return _gather(res.results)


# revision 6
# speedup vs baseline: 1.0687x; 1.0687x over previous
"""Trainium2 Bass kernel for nn_DifferentiableFDN.

Math: the module is linear in x, so
    out[b,t] = sum_j w_j * y_j[b,t],   w = (H^T alpha + beta)/16,
    y_j = first-order IIR of x with decay a_j.

Blocked-scan scheme (chunk length L=128, NCH=375 chunks per batch row).
The host pre-transposes x into XT[b] = (t=128, c=375) and un-transposes the
output. All matmul operands are bf16 (PSUM accumulates fp32); the chunk-carry
scan state stays fp32 inside the DVE. Per batch row b:
  - e  = P^T  @ XT   (16 x 375)   chunk-end state contributions, four
         matmuls in disjoint PE column quadrants run concurrently
  - S  : tensor_tensor_scan, S[c] = a_j^L S[c-1] + e[c], written bf16
         shifted (ssh[c] = S[c-1]), split into chained column halves so
         the A-half correction is never gated by the full scan
  - z  = MT^T @ XT   (128 x 375)  local Toeplitz part (start=True zeroes
         the whole bank row, so z is ONE matmul per bank)
  - z += Wc^T @ ssh  rank-16 carry correction, split into column halves
         (A = chunks 0:188, B = 188:375) in disjoint row quadrants so the
         A-half staging casts and output DMA launch while B still runs
  out[b, c*128+tp] = z[tp, c], cast bf16 into half-major staging tiles,
  A-half DMA on the sync queue / B-half on the scalar queue, host converts.

DMA plan (from ntff packet analysis): queue cost is ~6-9ns PER DESCRIPTOR
(one per SBUF partition row) regardless of row length up to ~3KB, plus
fixed SEQ ~0.6us + DGE ~0.65us + completion-sem straggle.  So ALL inputs
(x, the Toeplitz/carry weights, and the f32 scan multiplier byte-packed
into two bf16 columns) are fused host-side into ONE [128, 1774] bf16
tensor with 3548B rows = 128 descriptors total, split 72/56 across the
sync/scalar queues as exactly one DMA instruction each (the later-starting
scalar queue gets fewer rows).

PE clock: p-states top out at ~1.2GHz (0.83ns/col) for this kernel no
matter how long the warm-up streak runs (verified: a fully-bridged 6.7us
streak still streams 375-col matmuls in ~310ns).  The warm-up ladder still
pays for itself by lifting the clock from 0.65GHz before x lands.

Sharding: pure data-parallel, 4 batch rows per core x 8 cores.
"""
import numpy as np
import ml_dtypes

B, T = 32, 48000
D = 16
NCORES = 8
BL = B // NCORES            # 4 batch rows per core
L = 128                     # chunk length
NCH = T // L                # 375 chunks per batch row
NA = 188                    # A-half chunks (cols 0:188)
NB = NCH - NA               # B-half chunks (cols 188:375)
NIN = BL * NCH + 274        # fused input row length (x | MT | P | Wc | mlc)
XROWS_SYNC = 72             # fused-input partition rows on the sync queue
NWARM512 = 4                # warm-up ladder: big tiles first ...
NWARM256 = 2                # ... then 256-col quanta

_CACHE = {}


def _mirror_f32_params(log_kappa, alpha_raw, beta_raw, H):
    """Reference param math, f64 internally, rounded through f32 where the
    reference's f32 pipeline rounds."""
    sig = 1.0 / (1.0 + np.exp(-log_kappa.astype(np.float64)))
    sig32 = sig.astype(np.float32)
    kappa = (np.float32(1.0) + sig32 * np.float32(799.0)).astype(np.float32)
    inv = (np.float32(-1.0) / kappa).astype(np.float32)
    decays = np.exp(inv.astype(np.float64)).astype(np.float32)
    decays = np.clip(decays, 0.0, 0.9999).astype(np.float64)
    alpha = (1.0 / (1.0 + np.exp(-alpha_raw.astype(np.float64))))
    beta = (1.0 / (1.0 + np.exp(-beta_raw.astype(np.float64))))
    alpha = alpha.astype(np.float32).astype(np.float64)
    beta = beta.astype(np.float32).astype(np.float64)
    w = (H.astype(np.float64).T @ alpha + beta) / np.float64(D)
    return decays, w


def _tables(decays, w):
    delta = np.arange(L)
    pows = decays[None, :] ** delta[:, None]                   # [L, D] a_j^d
    h = pows @ w                                               # h[d]
    MT = np.zeros((L, L))
    for t in range(L):
        MT[t, t:] = h[: L - t]                                 # MT[t,tp]=h[tp-t]
    P = decays[None, :] ** (L - 1 - delta[:, None])            # [L, D]
    Wc = w[:, None] * decays[:, None] ** (delta[None, :] + 1)  # [D, L]
    bf = ml_dtypes.bfloat16
    # cc = [MT | P | Wc-replicated | mlc-bitcast] (128 x 274) bf16, appended
    # to the x rows host-side.  The 4 batch rows' chunk-end states live at
    # PSUM partition offsets 0/32/64/96 (the only legal PE output tile
    # positions), so the corr weights Wc and the scan multiplier mlc (f32,
    # byte-packed into bf16 cols 272:274 -- the scan state is fp32) are
    # replicated at those offsets.
    cc = np.zeros((L, 274), dtype=bf)
    cc[:, 0:128] = MT.astype(bf)
    cc[:, 128:144] = P.astype(bf)
    mlc = np.zeros((L,), dtype=np.float32)
    for b in range(BL):
        cc[32 * b:32 * b + D, 144:272] = Wc.astype(bf)
        mlc[32 * b:32 * b + D] = (decays ** L).astype(np.float32)
    cc[:, 272:274] = mlc.view(np.uint16).reshape(L, 2).view(bf)
    return cc


def _body(tc, oa_ap, ob_ap, xin_ap):
    from concourse import mybir
    from contextlib import ExitStack

    nc = tc.nc
    f32 = mybir.dt.float32
    bf16 = mybir.dt.bfloat16

    with ExitStack() as ctx:
        xtp = ctx.enter_context(tc.tile_pool(name="xt", bufs=1))
        sshp = ctx.enter_context(tc.tile_pool(name="sshp", bufs=1))
        stgp = ctx.enter_context(tc.tile_pool(name="stg", bufs=1))
        epp = ctx.enter_context(tc.tile_pool(name="e_ps", bufs=1, space="PSUM"))
        zpp = ctx.enter_context(tc.tile_pool(name="z_ps", bufs=1, space="PSUM"))

        xin = xtp.tile([L, NIN], bf16, tag="xin")
        xt = [xin[:, b * NCH:(b + 1) * NCH] for b in range(BL)]
        XB = BL * NCH
        mt_sb = xin[:, XB:XB + 128]
        p_sb = xin[:, XB + 128:XB + 144]
        wc_sb = xin[:, XB + 144:XB + 272]
        mlc_f32 = xin[:, XB + 272:XB + 274].bitcast(f32)   # [L, 1]
        ssh = sshp.tile([L, NCH], bf16, tag="ssh")
        e_all = epp.tile([L, NCH], f32, tag="e")

        # input: ONE fused DMA per HWDGE queue (72 rows sync / 56 scalar,
        # 3548B descriptors).
        nc.sync.dma_start(xin[0:XROWS_SYNC, :], xin_ap[0:XROWS_SYNC, :])
        nc.scalar.dma_start(xin[XROWS_SYNC:L, :], xin_ap[XROWS_SYNC:L, :])

        # scan writes cols 1..NCH-1; col 0 is the zero initial state
        nc.gpsimd.memset(ssh[:, 0:1], 0.0)

        # PE p-state warm-up: dependency-free ladder bridging the preamble
        # to the moment x lands (keeps the clock at ~1.2GHz for the real
        # matmuls instead of 0.65).
        warm_w = nc.const_aps.tensor(1.0, (L, L), bf16)
        warm_x = nc.const_aps.tensor(1.0, (L, 512), bf16)
        wpp = ctx.enter_context(tc.tile_pool(name="w_ps", bufs=1, space="PSUM"))
        w_ps = wpp.tile([L, 512], f32, tag="wps")
        for _ in range(NWARM512):
            nc.tensor.matmul(w_ps[:, :], lhsT=warm_w, rhs=warm_x,
                             start=True, stop=True)
        for _ in range(NWARM256):
            nc.tensor.matmul(w_ps[:, 0:256], lhsT=warm_w, rhs=warm_x[:, 0:256],
                             start=True, stop=True)

        # chunk-end states: 4 matmuls, same stationary P, partition-offset
        # writes (tile positions 0/32/64/96) into one stacked PSUM tile;
        # disjoint column quadrants let all four run concurrently on the PE
        for b in range(BL):
            nc.tensor.matmul(e_all[32 * b:32 * b + D, :], lhsT=p_sb,
                             rhs=xt[b], start=True, stop=True,
                             skip_group_check=True, tile_position=(0, 32 * b))

        # carry scan for all 4 batch rows at once (DVE cost is per-column),
        # split into chained halves; fp32 state internally, bf16 output.
        # The gap partitions carry garbage that nothing reads.
        nc.vector.tensor_tensor_scan(
            ssh[:, 1:NA], data0=mlc_f32[:, 0:1].broadcast_to((L, NA - 1)),
            data1=e_all[:, 0:NA - 1],
            initial=0.0, op0=mybir.AluOpType.mult, op1=mybir.AluOpType.add)
        nc.vector.tensor_tensor_scan(
            ssh[:, NA:NCH], data0=mlc_f32[:, 0:1].broadcast_to((L, NCH - NA)),
            data1=e_all[:, NA - 1:NCH - 1],
            initial=ssh[:, NA - 1:NA],
            op0=mybir.AluOpType.mult, op1=mybir.AluOpType.add)

        z = [zpp.tile([L, NCH], f32, tag=f"z{b}", name=f"z{b}")
             for b in range(BL)]
        for b in range(BL):
            nc.tensor.matmul(z[b][:, :], lhsT=mt_sb, rhs=xt[b][:, :],
                             start=True, stop=False, skip_group_check=True)
        # carry correction split into column halves so the A-half staging
        # casts (and output DMA) launch while the B-half still runs; each
        # half is 4 concurrent row-quadrant matmuls.
        for lo, hi, last in ((0, NA, False), (NA, NCH, True)):
            for b in range(BL):
                nc.tensor.matmul(z[b][:, lo:hi],
                                 lhsT=wc_sb[32 * b:32 * b + D, :],
                                 rhs=ssh[32 * b:32 * b + D, lo:hi],
                                 start=False, stop=last, skip_group_check=True,
                                 tile_position=(32 * b, 0))

        # half-major staging: stgA = [b0A|b1A|b2A|b3A], stgB likewise.
        # Casts split DVE (b0, b2) / Act (b1, b3); the A-half output DMA
        # goes on the idle sync engine the moment all A casts land, the
        # B-half issues from Act right after its last cast.
        stgA = stgp.tile([L, BL * NA], bf16, tag="stgA")
        stgB = stgp.tile([L, BL * NB], bf16, tag="stgB")
        for b in range(BL):
            dstA = stgA[:, b * NA:(b + 1) * NA]
            if b % 2:
                nc.scalar.copy(dstA, z[b][:, 0:NA])
            else:
                nc.vector.tensor_copy(dstA, z[b][:, 0:NA])
        for b in range(BL):
            dstB = stgB[:, b * NB:(b + 1) * NB]
            if b % 2:
                nc.scalar.copy(dstB, z[b][:, NA:NCH])
            else:
                nc.vector.tensor_copy(dstB, z[b][:, NA:NCH])
        nc.sync.dma_start(oa_ap[:, :], stgA[:, :])
        nc.scalar.dma_start(ob_ap[:, :], stgB[:, :])


def _build(num_devices=NCORES):
    import concourse.tile as tile
    from concourse import bacc, mybir

    bf16 = mybir.dt.bfloat16
    nc = bacc.Bacc("TRN2", target_bir_lowering=False, debug=False,
                   num_devices=num_devices)
    xin_ap = nc.dram_tensor("xin", [L, NIN], bf16, kind="ExternalInput").ap()
    # out halves: oa[tp, b*NA + c], ob[tp, b*NB + (c-NA)]
    oa_ap = nc.dram_tensor("oa", [L, BL * NA], bf16, kind="ExternalOutput").ap()
    ob_ap = nc.dram_tensor("ob", [L, BL * NB], bf16, kind="ExternalOutput").ap()

    with tile.TileContext(nc) as tc:
        _body(tc, oa_ap, ob_ap, xin_ap)
    nc.compile()
    return nc


def _in_maps(x, log_kappa, alpha_raw, beta_raw, H):
    decays, w = _mirror_f32_params(np.asarray(log_kappa), np.asarray(alpha_raw),
                                   np.asarray(beta_raw), np.asarray(H))
    cc = _tables(decays, w)
    bf = ml_dtypes.bfloat16
    x = np.asarray(x, dtype=np.float32)
    # host pre-transpose: (B, T) -> (B, L, NCH) bf16, then per-core fused
    # [x b-major | cc] rows of 3548B
    xt_all = x.reshape(B, NCH, L).transpose(0, 2, 1).astype(bf)  # (B, L, NCH)
    maps = []
    for c in range(NCORES):
        quad = xt_all[c * BL:(c + 1) * BL]           # (4, L, NCH)
        xin = np.empty((L, NIN), dtype=bf)
        xin[:, 0:BL * NCH] = quad.transpose(1, 0, 2).reshape(L, BL * NCH)
        xin[:, BL * NCH:] = cc
        maps.append({"xin": np.ascontiguousarray(xin)})
    return maps


def _gather(results):
    # oa (L, BL*NA) + ob (L, BL*NB) -> (BL, T), t = c*L + tp
    outs = []
    for c in range(NCORES):
        a = np.asarray(results[c]["oa"]).reshape(L, BL, NA)
        bb = np.asarray(results[c]["ob"]).reshape(L, BL, NB)
        arr = np.concatenate([a, bb], axis=2)        # (L, BL, NCH)
        outs.append(arr.transpose(1, 2, 0).reshape(BL, T))
    return np.concatenate(outs, axis=0).astype(np.float32)


def kernel(x, log_kappa, alpha_raw, beta_raw, H):
    from concourse import bass_utils

    if "nc" not in _CACHE:
        _CACHE["nc"] = _build()
    nc = _CACHE["nc"]
    maps = _in_maps(x, log_kappa, alpha_raw, beta_raw, H)
    res = bass_utils.run_bass_kernel_spmd(nc, maps, core_ids=list(range(NCORES)))
    return _gather(res.results)


# revision 7
# speedup vs baseline: 1.1039x; 1.0329x over previous
"""Trainium2 Bass kernel for nn_DifferentiableFDN.

Math: the module is linear in x, so
    out[b,t] = sum_j w_j * y_j[b,t],   w = (H^T alpha + beta)/16,
    y_j = first-order IIR of x with decay a_j.

Blocked-scan scheme (chunk length L=128, NCH=375 chunks per batch row).
The host pre-transposes x into XT[b] = (t=128, c=375) and un-transposes the
output. All matmul operands are bf16 (PSUM accumulates fp32); the chunk-carry
scan state stays fp32 inside the DVE. Per batch row b:
  - e  = P^T  @ XT   (16 x 375)   chunk-end state contributions, four
         matmuls in disjoint PE column quadrants run concurrently
  - S  : tensor_tensor_scan, S[c] = a_j^L S[c-1] + e[c], written bf16
         shifted (ssh[c] = S[c-1]), split into chained column halves so
         the correction is never gated by the full scan
  - z  = MT^T @ XT   (128 x 375)  local Toeplitz part (start=True zeroes
         the whole bank row, so z is ONE matmul per bank)
  - z += Wc^T @ ssh  rank-16 carry correction in column halves (the halves
         pipeline back-to-back on the PE at zero extra cost), four
         concurrent row-quadrant matmuls each
  out[b, c*128+tp] = z[tp, c], cast to bf16 into paired staging tiles
  (1500B DMA lines), two output DMAs, host converts to f32.

DMA plan (from ntff packet analysis): queue time is ~6-9ns per descriptor
(one per SBUF partition row, roughly independent of row bytes at 548 vs
1500B; ~3.5KB rows are byte-bound at ~165 B/ns so fusing buys nothing),
plus fixed SEQ ~0.6us + DGE ~0.65us per instruction and ~0.3-0.6us of
completion-sem straggle.  The sync queue's first packet beats scalar's by
~0.85us, so x rows split 152/104; the const pack (with the f32 scan
multiplier byte-packed into two bf16 columns) rides the gpsimd SWDGE
queue, which is slow (sems ~10.3us) but free — landing just before x.
Input is descriptor/byte floor-bound at ~10.2-10.3us on this part.

PE clock: p-states top out at ~1.2GHz (0.83ns/col) for this kernel no
matter how long the warm-up streak runs (verified: a fully-bridged 6.7us
streak still streams 375-col matmuls in ~310ns).  The warm-up ladder still
pays for itself by lifting the clock from 0.65GHz before x lands.

Sharding: pure data-parallel, 4 batch rows per core x 8 cores.
"""
import numpy as np
import ml_dtypes

B, T = 32, 48000
D = 16
NCORES = 8
BL = B // NCORES            # 4 batch rows per core
L = 128                     # chunk length
NCH = T // L                # 375 chunks per batch row
NA = 188                    # A-half chunks for the scan/corr splits
NWARM512 = 5                # warm-up ladder: big tiles first ...
NWARM256 = 2                # ... then 256-col quanta
XSY = 24                    # rows of xt23 carried by the sync queue

_CACHE = {}


def _mirror_f32_params(log_kappa, alpha_raw, beta_raw, H):
    """Reference param math, f64 internally, rounded through f32 where the
    reference's f32 pipeline rounds."""
    sig = 1.0 / (1.0 + np.exp(-log_kappa.astype(np.float64)))
    sig32 = sig.astype(np.float32)
    kappa = (np.float32(1.0) + sig32 * np.float32(799.0)).astype(np.float32)
    inv = (np.float32(-1.0) / kappa).astype(np.float32)
    decays = np.exp(inv.astype(np.float64)).astype(np.float32)
    decays = np.clip(decays, 0.0, 0.9999).astype(np.float64)
    alpha = (1.0 / (1.0 + np.exp(-alpha_raw.astype(np.float64))))
    beta = (1.0 / (1.0 + np.exp(-beta_raw.astype(np.float64))))
    alpha = alpha.astype(np.float32).astype(np.float64)
    beta = beta.astype(np.float32).astype(np.float64)
    w = (H.astype(np.float64).T @ alpha + beta) / np.float64(D)
    return decays, w


def _tables(decays, w):
    delta = np.arange(L)
    pows = decays[None, :] ** delta[:, None]                   # [L, D] a_j^d
    h = pows @ w                                               # h[d]
    MT = np.zeros((L, L))
    for t in range(L):
        MT[t, t:] = h[: L - t]                                 # MT[t,tp]=h[tp-t]
    P = decays[None, :] ** (L - 1 - delta[:, None])            # [L, D]
    Wc = w[:, None] * decays[:, None] ** (delta[None, :] + 1)  # [D, L]
    bf = ml_dtypes.bfloat16
    # cc = [MT | P | Wc-replicated | mlc-bitcast] (128 x 274) bf16, one DMA.
    # The 4 batch rows' chunk-end states live at PSUM partition offsets
    # 0/32/64/96 (the only legal PE output tile positions), so the corr
    # weights Wc and the scan multiplier mlc (f32, byte-packed into bf16
    # cols 272:274 -- the scan state is fp32) are replicated at those offsets.
    cc = np.zeros((L, 274), dtype=bf)
    cc[:, 0:128] = MT.astype(bf)
    cc[:, 128:144] = P.astype(bf)
    mlc = np.zeros((L,), dtype=np.float32)
    for b in range(BL):
        cc[32 * b:32 * b + D, 144:272] = Wc.astype(bf)
        mlc[32 * b:32 * b + D] = (decays ** L).astype(np.float32)
    cc[:, 272:274] = mlc.view(np.uint16).reshape(L, 2).view(bf)
    return np.ascontiguousarray(cc)


def _body(tc, o_ap, x_ap, cc_ap):
    from concourse import mybir
    from contextlib import ExitStack

    nc = tc.nc
    f32 = mybir.dt.float32
    bf16 = mybir.dt.bfloat16

    with ExitStack() as ctx:
        const = ctx.enter_context(tc.tile_pool(name="const", bufs=1))
        xtp = ctx.enter_context(tc.tile_pool(name="xt", bufs=1))
        sshp = ctx.enter_context(tc.tile_pool(name="sshp", bufs=1))
        stgp = ctx.enter_context(tc.tile_pool(name="stg", bufs=1))
        epp = ctx.enter_context(tc.tile_pool(name="e_ps", bufs=1, space="PSUM"))
        zpp = ctx.enter_context(tc.tile_pool(name="z_ps", bufs=1, space="PSUM"))

        cc = const.tile([L, 274], bf16, tag="cc")
        # batch rows are PAIRED per SBUF tile: 1500B partition lines keep the
        # DMA queues at full rate (750B lines run at ~half throughput)
        xtq = [xtp.tile([L, 2 * NCH], bf16, tag=f"xt{q}", name=f"xt{q}")
               for q in range(2)]
        xt = [xtq[b // 2][:, (b % 2) * NCH:(b % 2 + 1) * NCH] for b in range(BL)]
        ssh = sshp.tile([L, NCH], bf16, tag="ssh")
        e_all = epp.tile([L, NCH], f32, tag="e")

        # input DMAs: x rows split 152 (sync) / 104 (scalar) to equalize
        # finish given the queues' start offsets; const pack on SWDGE.
        nc.sync.dma_start(xtq[0][:, :], x_ap[0:L, :])
        nc.sync.dma_start(xtq[1][0:XSY, :], x_ap[L:L + XSY, :])
        nc.scalar.dma_start(xtq[1][XSY:L, :], x_ap[L + XSY:2 * L, :])
        nc.gpsimd.dma_start(cc[:, :], cc_ap[:, :])

        # scan writes cols 1..NCH-1; col 0 is the zero initial state
        nc.gpsimd.memset(ssh[:, 0:1], 0.0)

        # PE p-state warm-up: dependency-free ladder bridging the preamble
        # to the moment x lands (keeps the clock at ~1.2GHz for the real
        # matmuls instead of 0.65).
        warm_w = nc.const_aps.tensor(1.0, (L, L), bf16)
        warm_x = nc.const_aps.tensor(1.0, (L, 512), bf16)
        wpp = ctx.enter_context(tc.tile_pool(name="w_ps", bufs=1, space="PSUM"))
        w_ps = wpp.tile([L, 512], f32, tag="wps")
        for _ in range(NWARM512):
            nc.tensor.matmul(w_ps[:, :], lhsT=warm_w, rhs=warm_x,
                             start=True, stop=True)
        for _ in range(NWARM256):
            nc.tensor.matmul(w_ps[:, 0:256], lhsT=warm_w, rhs=warm_x[:, 0:256],
                             start=True, stop=True)

        mt_sb, p_sb = cc[:, 0:128], cc[:, 128:144]
        mlc_f32 = cc[:, 272:274].bitcast(f32)    # [L, 1] scan multiplier

        # chunk-end states: 4 matmuls, same stationary P, partition-offset
        # writes (tile positions 0/32/64/96) into one stacked PSUM tile;
        # disjoint column quadrants let all four run concurrently on the PE
        for b in range(BL):
            nc.tensor.matmul(e_all[32 * b:32 * b + D, :], lhsT=p_sb,
                             rhs=xt[b], start=True, stop=True,
                             skip_group_check=True, tile_position=(0, 32 * b))

        # carry scan for all 4 batch rows at once (DVE cost is per-column),
        # split into chained halves so the corr halves are never gated by
        # the full-length scan; fp32 state internally, bf16 output. The gap
        # partitions carry garbage that nothing reads.
        nc.vector.tensor_tensor_scan(
            ssh[:, 1:NA], data0=mlc_f32[:, 0:1].broadcast_to((L, NA - 1)),
            data1=e_all[:, 0:NA - 1],
            initial=0.0, op0=mybir.AluOpType.mult, op1=mybir.AluOpType.add)
        nc.vector.tensor_tensor_scan(
            ssh[:, NA:NCH], data0=mlc_f32[:, 0:1].broadcast_to((L, NCH - NA)),
            data1=e_all[:, NA - 1:NCH - 1],
            initial=ssh[:, NA - 1:NA],
            op0=mybir.AluOpType.mult, op1=mybir.AluOpType.add)

        z = [zpp.tile([L, NCH], f32, tag=f"z{b}", name=f"z{b}")
             for b in range(BL)]
        for b in range(BL):
            nc.tensor.matmul(z[b][:, :], lhsT=mt_sb, rhs=xt[b][:, :],
                             start=True, stop=False, skip_group_check=True)
        # carry correction in column halves (pipelines at no PE cost, and
        # each half only waits its own scan half)
        for lo, hi, last in ((0, NA, False), (NA, NCH, True)):
            for b in range(BL):
                nc.tensor.matmul(z[b][:, lo:hi],
                                 lhsT=cc[32 * b:32 * b + D, 144:272],
                                 rhs=ssh[32 * b:32 * b + D, lo:hi],
                                 start=False, stop=last, skip_group_check=True,
                                 tile_position=(32 * b, 0))

        # staging is paired (1500B lines, 2 output DMAs); within a pair one
        # copy runs on the DVE and one on the Activation engine, so each
        # output DMA launches after the pair's FIRST round of casts
        stq = [stgp.tile([L, 2 * NCH], bf16, tag=f"stg{q}", name=f"stg{q}")
               for q in range(2)]
        for b in range(BL):
            dst = stq[b // 2][:, (b % 2) * NCH:(b % 2 + 1) * NCH]
            if b % 2:
                nc.scalar.copy(dst, z[b][:, :])
            else:
                nc.vector.tensor_copy(dst, z[b][:, :])
        nc.sync.dma_start(o_ap[:, 0:2 * NCH], stq[0][:, :])
        nc.scalar.dma_start(o_ap[:, 2 * NCH:4 * NCH], stq[1][:, :])


def _build(num_devices=NCORES):
    import concourse.tile as tile
    from concourse import bacc, mybir

    bf16 = mybir.dt.bfloat16
    nc = bacc.Bacc("TRN2", target_bir_lowering=False, debug=False,
                   num_devices=num_devices)
    # x rows 0..127 = queue 0 (b0|b1 column-paired), rows 128..255 = queue 1
    x_ap = nc.dram_tensor("x", [2 * L, 2 * NCH], bf16, kind="ExternalInput").ap()
    cc_ap = nc.dram_tensor("cc", [L, 274], bf16, kind="ExternalInput").ap()
    # out[tp, b*NCH + c]
    o_ap = nc.dram_tensor("out", [L, BL * NCH], bf16, kind="ExternalOutput").ap()

    with tile.TileContext(nc) as tc:
        _body(tc, o_ap, x_ap, cc_ap)
    nc.compile()
    return nc


def _in_maps(x, log_kappa, alpha_raw, beta_raw, H):
    decays, w = _mirror_f32_params(np.asarray(log_kappa), np.asarray(alpha_raw),
                                   np.asarray(beta_raw), np.asarray(H))
    cc = _tables(decays, w)
    bf = ml_dtypes.bfloat16
    x = np.asarray(x, dtype=np.float32)
    # host pre-transpose: (B, T) -> per-core (2*L, 2*NCH) with batch rows
    # column-paired per DMA queue, bf16
    xt_all = x.reshape(B, NCH, L).transpose(0, 2, 1).astype(bf)  # (B, L, NCH)
    maps = []
    for c in range(NCORES):
        quad = xt_all[c * BL:(c + 1) * BL]           # (4, L, NCH)
        xs = quad.reshape(2, 2, L, NCH).transpose(0, 2, 1, 3).reshape(
            2 * L, 2 * NCH)                          # row q*L+p, col b*NCH+c
        maps.append({"x": np.ascontiguousarray(xs), "cc": cc})
    return maps


def _gather(results):
    # out dram per core: (L, BL*NCH) = [tp, (b, c)] -> (BL, T), t = c*L + tp
    outs = []
    for c in range(NCORES):
        arr = np.asarray(results[c]["out"]).reshape(L, BL, NCH)
        outs.append(arr.transpose(1, 2, 0).reshape(BL, T))
    return np.concatenate(outs, axis=0).astype(np.float32)


def kernel(x, log_kappa, alpha_raw, beta_raw, H):
    from concourse import bass_utils

    if "nc" not in _CACHE:
        _CACHE["nc"] = _build()
    nc = _CACHE["nc"]
    maps = _in_maps(x, log_kappa, alpha_raw, beta_raw, H)
    res = bass_utils.run_bass_kernel_spmd(nc, maps, core_ids=list(range(NCORES)))
    return _gather(res.results)


# revision 10
# speedup vs baseline: 1.1571x; 1.0482x over previous
"""Trainium2 Bass kernel for nn_DifferentiableFDN.

Math: the module is linear in x, so
    out[b,t] = sum_j w_j * y_j[b,t],   w = (H^T alpha + beta)/16,
    y_j = first-order IIR of x with decay a_j.

Blocked-scan scheme (chunk length L=128, NCH=375 chunks per batch row).
The host pre-transposes x into XT[b] = (t=128, c=375) and un-transposes the
output. All matmul operands are bf16 (PSUM accumulates fp32); the chunk-carry
scan state stays fp32 inside the DVE. Per batch row b:
  - e  = P^T  @ XT   (16 x 375)   chunk-end state contributions, four
         matmuls in disjoint PE column quadrants run concurrently
  - S  : tensor_tensor_scan, S[c] = a_j^L S[c-1] + e[c], written bf16
         shifted (ssh[c] = S[c-1]), split into chained column halves so
         the correction is never gated by the full scan
  - z  = MT^T @ XT   (128 x 375)  local Toeplitz part (start=True zeroes
         the whole bank row, so z is ONE matmul per bank)
  - z += Wc^T @ ssh  rank-16 carry correction in column halves (the halves
         pipeline back-to-back on the PE at zero extra cost), four
         concurrent row-quadrant matmuls each
  out[b, c*128+tp] = z[tp, c], cast to bf16 into paired staging tiles
  (1500B DMA lines), two output DMAs, host converts to f32.

DMA plan (from ntff packet analysis): queue time is ~6-9ns per descriptor
(one per SBUF partition row, roughly independent of row bytes at 548 vs
1500B; ~3.5KB rows are byte-bound at ~165 B/ns so fusing buys nothing),
plus fixed SEQ ~0.6us + DGE ~0.65us per instruction and ~0.3-0.6us of
completion-sem straggle.  The sync queue's first packet beats scalar's by
~0.85us, so x rows split 152/104; the const pack (with the f32 scan
multiplier byte-packed into two bf16 columns) rides the gpsimd SWDGE
queue, which is slow (sems ~10.3us) but free — landing just before x.
Input is descriptor/byte floor-bound at ~10.2-10.3us on this part.

PE clock: p-states top out at ~1.2GHz (0.83ns/col) for this kernel no
matter how long the warm-up streak runs (verified: a fully-bridged 6.7us
streak still streams 375-col matmuls in ~310ns).  The warm-up ladder still
pays for itself by lifting the clock from 0.65GHz before x lands.

Sharding: pure data-parallel, 4 batch rows per core x 8 cores.
"""
import numpy as np
import ml_dtypes

B, T = 32, 48000
D = 16
NCORES = 8
BL = B // NCORES            # 4 batch rows per core
L = 128                     # chunk length
NCH = T // L                # 375 chunks per batch row
NA = 188                    # A-half chunks for the scan/corr splits
NWARM512 = 5                # warm-up ladder: big tiles first ...
NWARM256 = 2                # ... then 256-col quanta
XSY = 24                    # rows of xt23 carried by the sync queue

_CACHE = {}


def _mirror_f32_params(log_kappa, alpha_raw, beta_raw, H):
    """Reference param math, f64 internally, rounded through f32 where the
    reference's f32 pipeline rounds."""
    sig = 1.0 / (1.0 + np.exp(-log_kappa.astype(np.float64)))
    sig32 = sig.astype(np.float32)
    kappa = (np.float32(1.0) + sig32 * np.float32(799.0)).astype(np.float32)
    inv = (np.float32(-1.0) / kappa).astype(np.float32)
    decays = np.exp(inv.astype(np.float64)).astype(np.float32)
    decays = np.clip(decays, 0.0, 0.9999).astype(np.float64)
    alpha = (1.0 / (1.0 + np.exp(-alpha_raw.astype(np.float64))))
    beta = (1.0 / (1.0 + np.exp(-beta_raw.astype(np.float64))))
    alpha = alpha.astype(np.float32).astype(np.float64)
    beta = beta.astype(np.float32).astype(np.float64)
    w = (H.astype(np.float64).T @ alpha + beta) / np.float64(D)
    return decays, w


def _tables(decays, w):
    delta = np.arange(L)
    pows = decays[None, :] ** delta[:, None]                   # [L, D] a_j^d
    h = pows @ w                                               # h[d]
    MT = np.zeros((L, L))
    for t in range(L):
        MT[t, t:] = h[: L - t]                                 # MT[t,tp]=h[tp-t]
    P = decays[None, :] ** (L - 1 - delta[:, None])            # [L, D]
    Wc = w[:, None] * decays[:, None] ** (delta[None, :] + 1)  # [D, L]
    bf = ml_dtypes.bfloat16
    # cc = [MT | P | Wc-replicated | mlc-bitcast] (128 x 274) bf16, one DMA.
    # The 4 batch rows' chunk-end states live at PSUM partition offsets
    # 0/32/64/96 (the only legal PE output tile positions), so the corr
    # weights Wc and the scan multiplier mlc (f32, byte-packed into bf16
    # cols 272:274 -- the scan state is fp32) are replicated at those offsets.
    cc = np.zeros((L, 274), dtype=bf)
    cc[:, 0:128] = MT.astype(bf)
    cc[:, 128:144] = P.astype(bf)
    mlc = np.zeros((L,), dtype=np.float32)
    for b in range(BL):
        cc[32 * b:32 * b + D, 144:272] = Wc.astype(bf)
        mlc[32 * b:32 * b + D] = (decays ** L).astype(np.float32)
    cc[:, 272:274] = mlc.view(np.uint16).reshape(L, 2).view(bf)
    return np.ascontiguousarray(cc)


def _body(tc, o_ap, x_ap, cc_ap):
    from concourse import mybir
    from contextlib import ExitStack

    nc = tc.nc
    f32 = mybir.dt.float32
    bf16 = mybir.dt.bfloat16

    with ExitStack() as ctx:
        const = ctx.enter_context(tc.tile_pool(name="const", bufs=1))
        xtp = ctx.enter_context(tc.tile_pool(name="xt", bufs=1))
        sshp = ctx.enter_context(tc.tile_pool(name="sshp", bufs=1))
        stgp = ctx.enter_context(tc.tile_pool(name="stg", bufs=1))
        epp = ctx.enter_context(tc.tile_pool(name="e_ps", bufs=1, space="PSUM"))
        zpp = ctx.enter_context(tc.tile_pool(name="z_ps", bufs=1, space="PSUM"))

        cc = const.tile([L, 274], bf16, tag="cc")
        # batch rows are PAIRED per SBUF tile: 1500B partition lines keep the
        # DMA queues at full rate (750B lines run at ~half throughput)
        xtq = [xtp.tile([L, 2 * NCH], bf16, tag=f"xt{q}", name=f"xt{q}")
               for q in range(2)]
        xt = [xtq[b // 2][:, (b % 2) * NCH:(b % 2 + 1) * NCH] for b in range(BL)]
        ssh = sshp.tile([L, NCH], bf16, tag="ssh")
        e_all = epp.tile([L, NCH], f32, tag="e")

        # input DMAs: exactly ONE bulk transfer per HWDGE queue (any extra
        # DMA instruction on a queue delays its first packet by ~0.45us);
        # const pack on the parallel SWDGE queue.
        nc.sync.dma_start(xtq[0][:, :], x_ap[0:L, :])
        nc.scalar.dma_start(xtq[1][:, :], x_ap[L:2 * L, :])
        nc.gpsimd.dma_start(cc[:, :], cc_ap[:, :])

        # scan writes cols 1..NCH-1; col 0 is the zero initial state
        nc.gpsimd.memset(ssh[:, 0:1], 0.0)

        # PE p-state warm-up: dependency-free ladder bridging the preamble
        # to the moment x lands (keeps the clock at ~1.2GHz for the real
        # matmuls instead of 0.65).
        warm_w = nc.const_aps.tensor(1.0, (L, L), bf16)
        warm_x = nc.const_aps.tensor(1.0, (L, 512), bf16)
        wpp = ctx.enter_context(tc.tile_pool(name="w_ps", bufs=1, space="PSUM"))
        w_ps = wpp.tile([L, 512], f32, tag="wps")
        for _ in range(NWARM512):
            nc.tensor.matmul(w_ps[:, :], lhsT=warm_w, rhs=warm_x,
                             start=True, stop=True)
        for _ in range(NWARM256):
            nc.tensor.matmul(w_ps[:, 0:256], lhsT=warm_w, rhs=warm_x[:, 0:256],
                             start=True, stop=True)

        mt_sb, p_sb = cc[:, 0:128], cc[:, 128:144]
        mlc_f32 = cc[:, 272:274].bitcast(f32)    # [L, 1] scan multiplier

        # chunk-end states: 4 matmuls, same stationary P, partition-offset
        # writes (tile positions 0/32/64/96) into one stacked PSUM tile;
        # disjoint column quadrants let all four run concurrently on the PE
        for b in range(BL):
            nc.tensor.matmul(e_all[32 * b:32 * b + D, :], lhsT=p_sb,
                             rhs=xt[b], start=True, stop=True,
                             skip_group_check=True, tile_position=(0, 32 * b))

        # ONE carry scan for all 4 batch rows (DVE cost is per-column, not
        # per-partition; splitting it costs more in instruction overhead +
        # sem hops than it buys); fp32 state internally, bf16 output. The
        # gap partitions carry garbage that nothing reads.
        nc.vector.tensor_tensor_scan(
            ssh[:, 1:NCH], data0=mlc_f32[:, 0:1].broadcast_to((L, NCH - 1)),
            data1=e_all[:, 0:NCH - 1],
            initial=0.0, op0=mybir.AluOpType.mult, op1=mybir.AluOpType.add)

        z = [zpp.tile([L, NCH], f32, tag=f"z{b}", name=f"z{b}")
             for b in range(BL)]
        for b in range(BL):
            nc.tensor.matmul(z[b][:, :], lhsT=mt_sb, rhs=xt[b][:, :],
                             start=True, stop=False, skip_group_check=True)
        for b in range(BL):
            nc.tensor.matmul(z[b][:, :], lhsT=cc[32 * b:32 * b + D, 144:272],
                             rhs=ssh[32 * b:32 * b + D, :],
                             start=False, stop=True, skip_group_check=True,
                             tile_position=(32 * b, 0))

        # staging is paired (1500B lines, 2 output DMAs); within a pair one
        # copy runs on the DVE and one on the Activation engine, so each
        # output DMA launches after the pair's FIRST round of casts
        stq = [stgp.tile([L, 2 * NCH], bf16, tag=f"stg{q}", name=f"stg{q}")
               for q in range(2)]
        for b in range(BL):
            dst = stq[b // 2][:, (b % 2) * NCH:(b % 2 + 1) * NCH]
            if b % 2:
                nc.scalar.copy(dst, z[b][:, :])
            else:
                nc.vector.tensor_copy(dst, z[b][:, :])
        nc.sync.dma_start(o_ap[:, 0:2 * NCH], stq[0][:, :])
        nc.scalar.dma_start(o_ap[:, 2 * NCH:4 * NCH], stq[1][:, :])


def _build(num_devices=NCORES):
    import concourse.tile as tile
    from concourse import bacc, mybir

    bf16 = mybir.dt.bfloat16
    nc = bacc.Bacc("TRN2", target_bir_lowering=False, debug=False,
                   num_devices=num_devices)
    # x rows 0..127 = queue 0 (b0|b1 column-paired), rows 128..255 = queue 1
    x_ap = nc.dram_tensor("x", [2 * L, 2 * NCH], bf16, kind="ExternalInput").ap()
    cc_ap = nc.dram_tensor("cc", [L, 274], bf16, kind="ExternalInput").ap()
    # out[tp, b*NCH + c]
    o_ap = nc.dram_tensor("out", [L, BL * NCH], bf16, kind="ExternalOutput").ap()

    with tile.TileContext(nc) as tc:
        _body(tc, o_ap, x_ap, cc_ap)
    nc.compile()
    return nc


def _in_maps(x, log_kappa, alpha_raw, beta_raw, H):
    decays, w = _mirror_f32_params(np.asarray(log_kappa), np.asarray(alpha_raw),
                                   np.asarray(beta_raw), np.asarray(H))
    cc = _tables(decays, w)
    bf = ml_dtypes.bfloat16
    x = np.asarray(x, dtype=np.float32)
    # host pre-transpose: (B, T) -> per-core (2*L, 2*NCH) with batch rows
    # column-paired per DMA queue, bf16
    xt_all = x.reshape(B, NCH, L).transpose(0, 2, 1).astype(bf)  # (B, L, NCH)
    maps = []
    for c in range(NCORES):
        quad = xt_all[c * BL:(c + 1) * BL]           # (4, L, NCH)
        xs = quad.reshape(2, 2, L, NCH).transpose(0, 2, 1, 3).reshape(
            2 * L, 2 * NCH)                          # row q*L+p, col b*NCH+c
        maps.append({"x": np.ascontiguousarray(xs), "cc": cc})
    return maps


def _gather(results):
    # out dram per core: (L, BL*NCH) = [tp, (b, c)] -> (BL, T), t = c*L + tp
    outs = []
    for c in range(NCORES):
        arr = np.asarray(results[c]["out"]).reshape(L, BL, NCH)
        outs.append(arr.transpose(1, 2, 0).reshape(BL, T))
    return np.concatenate(outs, axis=0).astype(np.float32)


def kernel(x, log_kappa, alpha_raw, beta_raw, H):
    from concourse import bass_utils

    if "nc" not in _CACHE:
        _CACHE["nc"] = _build()
    nc = _CACHE["nc"]
    maps = _in_maps(x, log_kappa, alpha_raw, beta_raw, H)
    res = bass_utils.run_bass_kernel_spmd(nc, maps, core_ids=list(range(NCORES)))
    return _gather(res.results)


# revision 17
# speedup vs baseline: 1.2162x; 1.0510x over previous
"""Trainium2 Bass kernel for nn_DifferentiableFDN.

Math: the module is linear in x, so
    out[b,t] = sum_j w_j * y_j[b,t],   w = (H^T alpha + beta)/16,
    y_j = first-order IIR of x with decay a_j.

Blocked-scan scheme (chunk length L=128, NCH=375 chunks per batch row).
The host pre-transposes x into XT[b] = (t=128, c=375) and un-transposes the
output. All matmul operands are bf16 (PSUM accumulates fp32); the chunk-carry
scan state stays fp32 inside the DVE. Per batch row b:
  - e  = P^T  @ XT   (16 x 375)   chunk-end state contributions, four
         matmuls in disjoint PE column quadrants run concurrently
  - S  : tensor_tensor_scan, S[c] = a_j^L S[c-1] + e[c], written bf16
         shifted (ssh[c] = S[c-1]), split into chained column halves so
         the correction is never gated by the full scan
  - z  = MT^T @ XT   (128 x 375)  local Toeplitz part (start=True zeroes
         the whole bank row, so z is ONE matmul per bank)
  - z += Wc^T @ ssh  rank-16 carry correction in column halves (the halves
         pipeline back-to-back on the PE at zero extra cost), four
         concurrent row-quadrant matmuls each
  out[b, c*128+tp] = z[tp, c], cast to bf16 into paired staging tiles
  (1500B DMA lines), two output DMAs, host converts to f32.

DMA plan (from ntff packet analysis): queue time is ~6-9ns per descriptor
(one per SBUF partition row, roughly independent of row bytes at 548 vs
1500B; ~3.5KB rows are byte-bound at ~165 B/ns so fusing buys nothing),
plus fixed SEQ ~0.6us + DGE ~0.65us per instruction and ~0.3-0.6us of
completion-sem straggle.  The sync queue's first packet beats scalar's by
~0.85us, so x rows split 152/104; the const pack (with the f32 scan
multiplier byte-packed into two bf16 columns) rides the gpsimd SWDGE
queue, which is slow (sems ~10.3us) but free — landing just before x.
Input is descriptor/byte floor-bound at ~10.2-10.3us on this part.

PE clock: p-states top out at ~1.2GHz (0.83ns/col) for this kernel no
matter how long the warm-up streak runs (verified: a fully-bridged 6.7us
streak still streams 375-col matmuls in ~310ns).  The warm-up ladder still
pays for itself by lifting the clock from 0.65GHz before x lands.

Sharding: pure data-parallel, 4 batch rows per core x 8 cores.
"""
import numpy as np
import ml_dtypes

B, T = 32, 48000
D = 16
NCORES = 8
BL = B // NCORES            # 4 batch rows per core
L = 128                     # chunk length
NCH = T // L                # 375 chunks per batch row
NA = 188                    # A-half chunks for the scan/corr splits
NWARM512 = 6                # warm-up ladder: big tiles first ...
NWARM256 = 2                # ... then 256-col quanta
XSY = 24                    # rows of xt23 carried by the sync queue

_CACHE = {}


def _mirror_f32_params(log_kappa, alpha_raw, beta_raw, H):
    """Reference param math, f64 internally, rounded through f32 where the
    reference's f32 pipeline rounds."""
    sig = 1.0 / (1.0 + np.exp(-log_kappa.astype(np.float64)))
    sig32 = sig.astype(np.float32)
    kappa = (np.float32(1.0) + sig32 * np.float32(799.0)).astype(np.float32)
    inv = (np.float32(-1.0) / kappa).astype(np.float32)
    decays = np.exp(inv.astype(np.float64)).astype(np.float32)
    decays = np.clip(decays, 0.0, 0.9999).astype(np.float64)
    alpha = (1.0 / (1.0 + np.exp(-alpha_raw.astype(np.float64))))
    beta = (1.0 / (1.0 + np.exp(-beta_raw.astype(np.float64))))
    alpha = alpha.astype(np.float32).astype(np.float64)
    beta = beta.astype(np.float32).astype(np.float64)
    w = (H.astype(np.float64).T @ alpha + beta) / np.float64(D)
    return decays, w


def _tables(decays, w):
    delta = np.arange(L)
    pows = decays[None, :] ** delta[:, None]                   # [L, D] a_j^d
    h = pows @ w                                               # h[d]
    MT = np.zeros((L, L))
    for t in range(L):
        MT[t, t:] = h[: L - t]                                 # MT[t,tp]=h[tp-t]
    P = decays[None, :] ** (L - 1 - delta[:, None])            # [L, D]
    Wc = w[:, None] * decays[:, None] ** (delta[None, :] + 1)  # [D, L]
    bf = ml_dtypes.bfloat16
    # cc = [MT | P | Wc-replicated | mlc-bitcast] (128 x 274) bf16, one DMA.
    # The 4 batch rows' chunk-end states live at PSUM partition offsets
    # 0/32/64/96 (the only legal PE output tile positions), so the corr
    # weights Wc and the scan multiplier mlc (f32, byte-packed into bf16
    # cols 272:274 -- the scan state is fp32) are replicated at those offsets.
    cc = np.zeros((L, 274), dtype=bf)
    cc[:, 0:128] = MT.astype(bf)
    cc[:, 128:144] = P.astype(bf)
    mlc = np.zeros((L,), dtype=np.float32)
    for b in range(BL):
        cc[32 * b:32 * b + D, 144:272] = Wc.astype(bf)
        mlc[32 * b:32 * b + D] = (decays ** L).astype(np.float32)
    cc[:, 272:274] = mlc.view(np.uint16).reshape(L, 2).view(bf)
    return np.ascontiguousarray(cc)


def _body(tc, o_ap, x_ap, cc_ap):
    from concourse import mybir
    from contextlib import ExitStack

    nc = tc.nc
    f32 = mybir.dt.float32
    bf16 = mybir.dt.bfloat16

    with ExitStack() as ctx:
        const = ctx.enter_context(tc.tile_pool(name="const", bufs=1))
        xtp = ctx.enter_context(tc.tile_pool(name="xt", bufs=1))
        sshp = ctx.enter_context(tc.tile_pool(name="sshp", bufs=1))
        stgp = ctx.enter_context(tc.tile_pool(name="stg", bufs=1))
        epp = ctx.enter_context(tc.tile_pool(name="e_ps", bufs=1, space="PSUM"))
        zpp = ctx.enter_context(tc.tile_pool(name="z_ps", bufs=1, space="PSUM"))

        cc = const.tile([L, 274], bf16, tag="cc")
        # batch rows are PAIRED per SBUF tile: 1500B partition lines keep the
        # DMA queues at full rate (750B lines run at ~half throughput)
        xtq = [xtp.tile([L, 2 * NCH], bf16, tag=f"xt{q}", name=f"xt{q}")
               for q in range(2)]
        xt = [xtq[b // 2][:, (b % 2) * NCH:(b % 2 + 1) * NCH] for b in range(BL)]
        ssh = sshp.tile([L, NCH], bf16, tag="ssh")
        e_all = epp.tile([L, NCH], f32, tag="e")

        # input DMAs: exactly ONE bulk transfer per HWDGE queue (any extra
        # DMA instruction on a queue delays its first packet by ~0.45us);
        # const pack on the parallel SWDGE queue.
        nc.sync.dma_start(xtq[0][:, :], x_ap[0:L, :])
        nc.scalar.dma_start(xtq[1][:, :], x_ap[L:2 * L, :])
        nc.gpsimd.dma_start(cc[:, :], cc_ap[:, :])

        # scan writes cols 1..NCH-1; col 0 is the zero initial state
        nc.gpsimd.memset(ssh[:, 0:1], 0.0)

        # PE p-state warm-up: dependency-free ladder bridging the preamble
        # to the moment x lands (keeps the clock at ~1.2GHz for the real
        # matmuls instead of 0.65).
        warm_w = nc.const_aps.tensor(1.0, (L, L), bf16)
        warm_x = nc.const_aps.tensor(1.0, (L, 512), bf16)
        wpp = ctx.enter_context(tc.tile_pool(name="w_ps", bufs=1, space="PSUM"))
        w_ps = wpp.tile([L, 512], f32, tag="wps")
        for _ in range(NWARM512):
            nc.tensor.matmul(w_ps[:, :], lhsT=warm_w, rhs=warm_x,
                             start=True, stop=True)
        for _ in range(NWARM256):
            nc.tensor.matmul(w_ps[:, 0:256], lhsT=warm_w, rhs=warm_x[:, 0:256],
                             start=True, stop=True)

        mt_sb, p_sb = cc[:, 0:128], cc[:, 128:144]
        mlc_f32 = cc[:, 272:274].bitcast(f32)    # [L, 1] scan multiplier

        # chunk-end states: 4 matmuls, same stationary P, partition-offset
        # writes (tile positions 0/32/64/96) into one stacked PSUM tile;
        # disjoint column quadrants let all four run concurrently on the PE
        for b in range(BL):
            nc.tensor.matmul(e_all[32 * b:32 * b + D, :], lhsT=p_sb,
                             rhs=xt[b], start=True, stop=True,
                             skip_group_check=True, tile_position=(0, 32 * b))

        # ONE carry scan for all 4 batch rows (DVE cost is per-column, not
        # per-partition; splitting it costs more in instruction overhead +
        # sem hops than it buys); fp32 state internally, bf16 output. The
        # gap partitions carry garbage that nothing reads.
        nc.vector.tensor_tensor_scan(
            ssh[:, 1:NCH], data0=mlc_f32[:, 0:1].broadcast_to((L, NCH - 1)),
            data1=e_all[:, 0:NCH - 1],
            initial=0.0, op0=mybir.AluOpType.mult, op1=mybir.AluOpType.add)

        z = [zpp.tile([L, NCH], f32, tag=f"z{b}", name=f"z{b}")
             for b in range(BL)]
        for b in range(BL):
            nc.tensor.matmul(z[b][:, :], lhsT=mt_sb, rhs=xt[b][:, :],
                             start=True, stop=False, skip_group_check=True)
        for b in range(BL):
            nc.tensor.matmul(z[b][:, :], lhsT=cc[32 * b:32 * b + D, 144:272],
                             rhs=ssh[32 * b:32 * b + D, :],
                             start=False, stop=True, skip_group_check=True,
                             tile_position=(32 * b, 0))

        # staging is paired (1500B lines, 2 output DMAs); within a pair one
        # copy runs on the DVE and one on the Activation engine, so each
        # output DMA launches after the pair's FIRST round of casts
        stq = [stgp.tile([L, 2 * NCH], bf16, tag=f"stg{q}", name=f"stg{q}")
               for q in range(2)]
        for b in range(BL):
            dst = stq[b // 2][:, (b % 2) * NCH:(b % 2 + 1) * NCH]
            if b % 2:
                nc.scalar.copy(dst, z[b][:, :])
            else:
                nc.vector.tensor_copy(dst, z[b][:, :])
        nc.sync.dma_start(o_ap[:, 0:2 * NCH], stq[0][:, :])
        nc.scalar.dma_start(o_ap[:, 2 * NCH:4 * NCH], stq[1][:, :])


def _build(num_devices=NCORES):
    import concourse.tile as tile
    from concourse import bacc, mybir

    bf16 = mybir.dt.bfloat16
    nc = bacc.Bacc("TRN2", target_bir_lowering=False, debug=False,
                   num_devices=num_devices)
    # x rows 0..127 = queue 0 (b0|b1 column-paired), rows 128..255 = queue 1
    x_ap = nc.dram_tensor("x", [2 * L, 2 * NCH], bf16, kind="ExternalInput").ap()
    cc_ap = nc.dram_tensor("cc", [L, 274], bf16, kind="ExternalInput").ap()
    # out[tp, b*NCH + c]
    o_ap = nc.dram_tensor("out", [L, BL * NCH], bf16, kind="ExternalOutput").ap()

    with tile.TileContext(nc) as tc:
        _body(tc, o_ap, x_ap, cc_ap)
    nc.compile()
    return nc


def _in_maps(x, log_kappa, alpha_raw, beta_raw, H):
    decays, w = _mirror_f32_params(np.asarray(log_kappa), np.asarray(alpha_raw),
                                   np.asarray(beta_raw), np.asarray(H))
    cc = _tables(decays, w)
    bf = ml_dtypes.bfloat16
    x = np.asarray(x, dtype=np.float32)
    # host pre-transpose: (B, T) -> per-core (2*L, 2*NCH) with batch rows
    # column-paired per DMA queue, bf16
    xt_all = x.reshape(B, NCH, L).transpose(0, 2, 1).astype(bf)  # (B, L, NCH)
    maps = []
    for c in range(NCORES):
        quad = xt_all[c * BL:(c + 1) * BL]           # (4, L, NCH)
        xs = quad.reshape(2, 2, L, NCH).transpose(0, 2, 1, 3).reshape(
            2 * L, 2 * NCH)                          # row q*L+p, col b*NCH+c
        maps.append({"x": np.ascontiguousarray(xs), "cc": cc})
    return maps


def _gather(results):
    # out dram per core: (L, BL*NCH) = [tp, (b, c)] -> (BL, T), t = c*L + tp
    outs = []
    for c in range(NCORES):
        arr = np.asarray(results[c]["out"]).reshape(L, BL, NCH)
        outs.append(arr.transpose(1, 2, 0).reshape(BL, T))
    return np.concatenate(outs, axis=0).astype(np.float32)


def kernel(x, log_kappa, alpha_raw, beta_raw, H):
    from concourse import bass_utils

    if "nc" not in _CACHE:
        _CACHE["nc"] = _build()
    nc = _CACHE["nc"]
    maps = _in_maps(x, log_kappa, alpha_raw, beta_raw, H)
    res = bass_utils.run_bass_kernel_spmd(nc, maps, core_ids=list(range(NCORES)))
    return _gather(res.results)
